# revision 46
# baseline (speedup 1.0000x reference)
"""Bass/Trainium2 kernel for nn_PidgeonNetLoss (segment-reduce discriminative loss).

Full inputs: y [8, 16, 512, 512] fp32, c [8, 1, 512, 512] int32 (labels 0..31).
Sharding: data-parallel over batch -> 1 sample per NeuronCore (8 cores).
Per-core: segment stats via one-hot matmul accumulation in PSUM, then tiny
hinge-loss finalization; per-sample (var, dis) means are combined on host.
"""

import numpy as np
from contextlib import ExitStack

import concourse.bass as bass
import concourse.bacc as bacc
import concourse.tile as tile
import concourse.mybir as mybir
from concourse._compat import with_exitstack
from concourse.bass_utils import run_bass_kernel_spmd

# ---- problem constants (hardcoded, kernel.py must be self-contained) ----
B = 8
NCH = 16          # channels D
H = W = 512
P = H * W         # 262144 pixels per sample
K = 32            # labels
QP = 128          # partitions
FT = P // QP      # 2048 pixels per partition-row
# graded block schedules: small first blocks shorten the pipeline fill
Y_BLOCKS = [128, 128] + [256] * 7
OH_BLOCKS = [128, 384] + [512] * 3
assert sum(Y_BLOCKS) == FT and sum(OH_BLOCKS) == FT
VAR_MARGIN = 1.0
DIST_MARGIN = 3.0
EPS = 1e-12

FP32 = mybir.dt.float32
BF16 = mybir.dt.bfloat16
INT32 = mybir.dt.int32


@with_exitstack
def _pidgeon_kernel(ctx: ExitStack, tc: "tile.TileContext", repeat: int = 1):
    nc = tc.nc
    y_d = nc.dram_tensor("y", [NCH, P], FP32, kind="ExternalInput")
    c_d = nc.dram_tensor("c", [P], INT32, kind="ExternalInput")
    out_d = nc.dram_tensor("out", [1, 2], FP32, kind="ExternalOutput")
    for _rep in range(repeat):
        _pidgeon_body(ctx, tc, y_d, c_d, out_d)


def _pidgeon_body(ctx, tc, y_d, c_d, out_d):
    nc = tc.nc

    # view: y[ch, q*FT + f] -> [q, ch, f]
    y_r = y_d[:, :].rearrange("c (q f) -> q c f", q=QP)
    c_r = c_d[:].rearrange("(q f) -> q f", q=QP)

    stack = ExitStack()
    const_pool = stack.enter_context(tc.tile_pool(name="const", bufs=1))
    yall_pool = stack.enter_context(tc.tile_pool(name="yall", bufs=4))
    oh_pool = stack.enter_context(tc.tile_pool(name="oh", bufs=2))
    fin_pool = stack.enter_context(tc.tile_pool(name="fin", bufs=1))
    psum_pool = stack.enter_context(
        tc.tile_pool(name="psum", bufs=1, space=bass.MemorySpace.PSUM)
    )

    # labels for the whole sample: [128, 2048] int32 -> bf16.
    # Loaded in two pieces so the first one-hot block starts early.
    C0 = OH_BLOCKS[0]
    c_i = const_pool.tile([QP, FT], INT32)
    nc.sync.dma_start(c_i[:, 0:C0], c_r[:, 0:C0])
    nc.sync.dma_start(c_i[:, C0:FT], c_r[:, C0:FT])
    c_bf = const_pool.tile([QP, FT], BF16)
    nc.vector.tensor_copy(c_bf[:, 0:C0], c_i[:, 0:C0])
    nc.vector.tensor_copy(c_bf[:, C0:FT], c_i[:, C0:FT])

    acc = psum_pool.tile([K, 33], FP32)

    y_blocks = list(Y_BLOCKS)
    o0 = 0
    for osz in OH_BLOCKS:
        oh = oh_pool.tile([QP, K, osz], BF16, tag="oh")
        for k in range(K):
            nc.vector.tensor_scalar(
                oh[:, k, :],
                c_bf[:, o0 : o0 + osz],
                float(k),
                None,
                mybir.AluOpType.is_equal,
            )
        # y blocks + matmuls covering [o0, o0+osz)
        y0 = o0
        while y0 < o0 + osz:
            fsz = y_blocks.pop(0)
            y_all = yall_pool.tile([QP, 33, fsz], BF16, tag="yall")
            nc.gpsimd.dma_start(y_all[:, 0:NCH, :], y_r[:, :, y0 : y0 + fsz])
            nc.scalar.square(y_all[:, NCH : 2 * NCH, :], y_all[:, 0:NCH, :])
            nc.vector.memset(y_all[:, 2 * NCH, :], 1.0)
            for f in range(fsz):
                g = y0 + f
                nc.tensor.matmul(
                    acc[:],
                    oh[:, :, g - o0],
                    y_all[:, :, f],
                    start=(g == 0),
                    stop=(g == FT - 1),
                )
            y0 += fsz
        o0 += osz

    # ---------------- finalization ----------------
    eps_k = fin_pool.tile([K, 1], FP32)
    nc.vector.memset(eps_k[:], EPS)

    st = fin_pool.tile([K, 33], FP32)
    nc.vector.tensor_copy(st[:], acc[:])

    counts = st[:, 32:33]
    rc = fin_pool.tile([K, 1], FP32)
    nc.vector.reciprocal(rc[:], counts)

    centroid = fin_pool.tile([K, NCH], FP32)
    nc.vector.tensor_scalar(
        centroid[:], st[:, 0:NCH], rc[:], None, mybir.AluOpType.mult
    )

    # cn2 = sum_d centroid^2  [K,1]
    c2tmp = fin_pool.tile([K, NCH], FP32)
    cn2 = fin_pool.tile([K, 1], FP32)
    nc.vector.tensor_tensor(c2tmp[:], centroid[:], centroid[:], mybir.AluOpType.mult)
    nc.vector.tensor_reduce(
        cn2[:], c2tmp[:], axis=mybir.AxisListType.X, op=mybir.AluOpType.add
    )

    # sqy = sum_d sumsq  [K,1]
    sqy = fin_pool.tile([K, 1], FP32)
    nc.vector.tensor_reduce(
        sqy[:], st[:, NCH : 2 * NCH], axis=mybir.AxisListType.X, op=mybir.AluOpType.add
    )

    # sq = max(sqy - counts*cn2, 0); fro = sqrt(sq + eps)
    t0 = fin_pool.tile([K, 1], FP32)
    nc.vector.tensor_scalar(t0[:], cn2[:], counts, None, mybir.AluOpType.mult)
    sq = fin_pool.tile([K, 1], FP32)
    nc.vector.tensor_tensor(sq[:], sqy[:], t0[:], mybir.AluOpType.subtract)
    nc.vector.tensor_scalar_max(sq[:], sq[:], 0.0)
    fro = fin_pool.tile([K, 1], FP32)
    nc.scalar.activation(
        fro[:], sq[:], mybir.ActivationFunctionType.Sqrt, bias=eps_k[:]
    )

    # v = relu(fro - margin)^2 ; var_sum = sum_k v
    v = fin_pool.tile([K, 1], FP32)
    nc.vector.tensor_scalar(
        v[:], fro[:], -VAR_MARGIN, 0.0, mybir.AluOpType.add, mybir.AluOpType.max
    )
    nc.vector.tensor_tensor(v[:], v[:], v[:], mybir.AluOpType.mult)
    ones_k = fin_pool.tile([K, 1], FP32)
    nc.vector.memset(ones_k[:], 1.0)
    vsum_ps = psum_pool.tile([1, 1], FP32)
    nc.tensor.matmul(vsum_ps[:], ones_k[:], v[:], start=True, stop=True)
    vsum = fin_pool.tile([1, 1], FP32)
    nc.vector.tensor_copy(vsum[:], vsum_ps[:])

    # centroid^T via DVE 32x32 block transpose: [32,32] -> rows 0:16 hold [16,32]
    cw = fin_pool.tile([K, K], FP32)
    nc.vector.memset(cw[:], 0.0)
    nc.vector.tensor_copy(cw[:, 0:NCH], centroid[:])
    ct = fin_pool.tile([K, K], FP32)
    nc.vector.transpose(ct[:], cw[:])

    # diff = ct - roll(ct, 1, axis=free) over label axis (cols), rows 0:16
    diff = fin_pool.tile([NCH, K], FP32)
    nc.vector.tensor_tensor(
        diff[:, 1:K], ct[0:NCH, 1:K], ct[0:NCH, 0 : K - 1], mybir.AluOpType.subtract
    )
    nc.vector.tensor_tensor(
        diff[:, 0:1], ct[0:NCH, 0:1], ct[0:NCH, K - 1 : K], mybir.AluOpType.subtract
    )

    d2 = fin_pool.tile([NCH, K], FP32)
    nc.vector.tensor_tensor(d2[:], diff[:], diff[:], mybir.AluOpType.mult)
    dsq_ps = psum_pool.tile([1, K], FP32)
    nc.tensor.matmul(dsq_ps[:], ones_k[0:NCH, :], d2[:], start=True, stop=True)
    dsq = fin_pool.tile([1, K], FP32)
    nc.vector.tensor_copy(dsq[:], dsq_ps[:])

    dist = fin_pool.tile([1, K], FP32)
    nc.scalar.activation(
        dist[:], dsq[:], mybir.ActivationFunctionType.Sqrt, bias=eps_k[0:1, :]
    )
    # hinge = max(margin - dist, 0)^2 ; dis_sum = sum_k
    hg = fin_pool.tile([1, K], FP32)
    nc.vector.tensor_scalar(
        hg[:], dist[:], -1.0, DIST_MARGIN, mybir.AluOpType.mult, mybir.AluOpType.add
    )
    nc.vector.tensor_scalar_max(hg[:], hg[:], 0.0)
    nc.vector.tensor_tensor(hg[:], hg[:], hg[:], mybir.AluOpType.mult)
    dsum = fin_pool.tile([1, 1], FP32)
    nc.vector.tensor_reduce(
        dsum[:], hg[:], axis=mybir.AxisListType.X, op=mybir.AluOpType.add
    )

    out_sb = fin_pool.tile([1, 2], FP32)
    nc.vector.tensor_scalar_mul(out_sb[:, 0:1], vsum[:], 1.0 / K)
    nc.vector.tensor_scalar_mul(out_sb[:, 1:2], dsum[:], 1.0 / K)
    nc.sync.dma_start(out_d[:], out_sb[:])
    stack.close()


_NC_CACHE = {}


def _build(repeat: int = 1):
    if repeat not in _NC_CACHE:
        nc = bacc.Bacc(None, target_bir_lowering=False)
        with tile.TileContext(nc) as tc:
            _pidgeon_kernel(tc, repeat=repeat)
        nc.finalize()
        _NC_CACHE[repeat] = nc
    return _NC_CACHE[repeat]


def kernel(y: np.ndarray, c: np.ndarray) -> np.ndarray:
    assert y.shape == (B, NCH, H, W) and c.shape == (B, 1, H, W)
    nc = _build()
    in_maps = [
        {
            "y": np.ascontiguousarray(y[i].reshape(NCH, P), dtype=np.float32),
            "c": np.ascontiguousarray(c[i, 0].reshape(P), dtype=np.int32),
        }
        for i in range(B)
    ]
    last_err = None
    for _attempt in range(3):
        try:
            res = run_bass_kernel_spmd(nc, in_maps, list(range(B)))
            break
        except Exception as e:  # transient device errors: retry
            last_err = e
    else:
        raise last_err
    outs = np.stack([res.results[i]["out"].reshape(2) for i in range(B)])
    loss = outs[:, 0].mean() + outs[:, 1].mean()
    return np.float32(loss)


# revision 47
# speedup vs baseline: 1.0003x; 1.0003x over previous
"""Bass/Trainium2 kernel for nn_PidgeonNetLoss (segment-reduce discriminative loss).

Full inputs: y [8, 16, 512, 512] fp32, c [8, 1, 512, 512] int32 (labels 0..31).
Sharding: data-parallel over batch -> 1 sample per NeuronCore (8 cores).
Per-core: segment stats via one-hot matmul accumulation in PSUM, then tiny
hinge-loss finalization; per-sample (var, dis) means are combined on host.
"""

import numpy as np
from contextlib import ExitStack

import concourse.bass as bass
import concourse.bacc as bacc
import concourse.tile as tile
import concourse.mybir as mybir
from concourse._compat import with_exitstack
from concourse.bass_utils import run_bass_kernel_spmd

# ---- problem constants (hardcoded, kernel.py must be self-contained) ----
B = 8
NCH = 16          # channels D
H = W = 512
P = H * W         # 262144 pixels per sample
K = 32            # labels
QP = 128          # partitions
FT = P // QP      # 2048 pixels per partition-row
# graded block schedules: small first blocks shorten the pipeline fill,
# small tail blocks shorten the last square->matmul dependency edge
Y_BLOCKS = [128, 128] + [256] * 6 + [128, 128]
OH_BLOCKS = [128, 384] + [512] * 3
assert sum(Y_BLOCKS) == FT and sum(OH_BLOCKS) == FT
VAR_MARGIN = 1.0
DIST_MARGIN = 3.0
EPS = 1e-12

FP32 = mybir.dt.float32
BF16 = mybir.dt.bfloat16
INT32 = mybir.dt.int32


@with_exitstack
def _pidgeon_kernel(ctx: ExitStack, tc: "tile.TileContext", repeat: int = 1):
    nc = tc.nc
    y_d = nc.dram_tensor("y", [NCH, P], FP32, kind="ExternalInput")
    c_d = nc.dram_tensor("c", [P], INT32, kind="ExternalInput")
    out_d = nc.dram_tensor("out", [1, 2], FP32, kind="ExternalOutput")
    for _rep in range(repeat):
        _pidgeon_body(ctx, tc, y_d, c_d, out_d)


def _pidgeon_body(ctx, tc, y_d, c_d, out_d):
    nc = tc.nc

    # view: y[ch, q*FT + f] -> [q, ch, f]
    y_r = y_d[:, :].rearrange("c (q f) -> q c f", q=QP)
    c_r = c_d[:].rearrange("(q f) -> q f", q=QP)

    stack = ExitStack()
    const_pool = stack.enter_context(tc.tile_pool(name="const", bufs=1))
    yall_pool = stack.enter_context(tc.tile_pool(name="yall", bufs=4))
    oh_pool = stack.enter_context(tc.tile_pool(name="oh", bufs=2))
    fin_pool = stack.enter_context(tc.tile_pool(name="fin", bufs=1))
    psum_pool = stack.enter_context(
        tc.tile_pool(name="psum", bufs=1, space=bass.MemorySpace.PSUM)
    )

    # labels for the whole sample: [128, 2048] int32 -> bf16.
    # Loaded in two pieces so the first one-hot block starts early.
    C0 = OH_BLOCKS[0]
    c_i = const_pool.tile([QP, FT], INT32)
    nc.sync.dma_start(c_i[:, 0:C0], c_r[:, 0:C0])
    nc.sync.dma_start(c_i[:, C0:FT], c_r[:, C0:FT])
    c_bf = const_pool.tile([QP, FT], BF16)
    nc.vector.tensor_copy(c_bf[:, 0:C0], c_i[:, 0:C0])
    nc.vector.tensor_copy(c_bf[:, C0:FT], c_i[:, C0:FT])

    acc = psum_pool.tile([K, 33], FP32)

    y_blocks = list(Y_BLOCKS)
    o0 = 0
    for osz in OH_BLOCKS:
        oh = oh_pool.tile([QP, K, osz], BF16, tag="oh")
        for k in range(K):
            nc.vector.tensor_scalar(
                oh[:, k, :],
                c_bf[:, o0 : o0 + osz],
                float(k),
                None,
                mybir.AluOpType.is_equal,
            )
        # y blocks + matmuls covering [o0, o0+osz)
        y0 = o0
        while y0 < o0 + osz:
            fsz = y_blocks.pop(0)
            y_all = yall_pool.tile([QP, 33, fsz], BF16, tag="yall")
            nc.gpsimd.dma_start(y_all[:, 0:NCH, :], y_r[:, :, y0 : y0 + fsz])
            nc.scalar.square(y_all[:, NCH : 2 * NCH, :], y_all[:, 0:NCH, :])
            nc.vector.memset(y_all[:, 2 * NCH, :], 1.0)
            for f in range(fsz):
                g = y0 + f
                nc.tensor.matmul(
                    acc[:],
                    oh[:, :, g - o0],
                    y_all[:, :, f],
                    start=(g == 0),
                    stop=(g == FT - 1),
                )
            y0 += fsz
        o0 += osz

    # ---------------- finalization ----------------
    eps_k = fin_pool.tile([K, 1], FP32)
    nc.vector.memset(eps_k[:], EPS)

    st = fin_pool.tile([K, 33], FP32)
    nc.vector.tensor_copy(st[:], acc[:])

    counts = st[:, 32:33]
    rc = fin_pool.tile([K, 1], FP32)
    nc.vector.reciprocal(rc[:], counts)

    centroid = fin_pool.tile([K, NCH], FP32)
    nc.vector.tensor_scalar(
        centroid[:], st[:, 0:NCH], rc[:], None, mybir.AluOpType.mult
    )

    # cn2 = sum_d centroid^2  [K,1]
    c2tmp = fin_pool.tile([K, NCH], FP32)
    cn2 = fin_pool.tile([K, 1], FP32)
    nc.vector.tensor_tensor(c2tmp[:], centroid[:], centroid[:], mybir.AluOpType.mult)
    nc.vector.tensor_reduce(
        cn2[:], c2tmp[:], axis=mybir.AxisListType.X, op=mybir.AluOpType.add
    )

    # sqy = sum_d sumsq  [K,1]
    sqy = fin_pool.tile([K, 1], FP32)
    nc.vector.tensor_reduce(
        sqy[:], st[:, NCH : 2 * NCH], axis=mybir.AxisListType.X, op=mybir.AluOpType.add
    )

    # sq = max(sqy - counts*cn2, 0); fro = sqrt(sq + eps)
    t0 = fin_pool.tile([K, 1], FP32)
    nc.vector.tensor_scalar(t0[:], cn2[:], counts, None, mybir.AluOpType.mult)
    sq = fin_pool.tile([K, 1], FP32)
    nc.vector.tensor_tensor(sq[:], sqy[:], t0[:], mybir.AluOpType.subtract)
    nc.vector.tensor_scalar_max(sq[:], sq[:], 0.0)
    fro = fin_pool.tile([K, 1], FP32)
    nc.scalar.activation(
        fro[:], sq[:], mybir.ActivationFunctionType.Sqrt, bias=eps_k[:]
    )

    # v = relu(fro - margin)^2 ; var_sum = sum_k v
    v = fin_pool.tile([K, 1], FP32)
    nc.vector.tensor_scalar(
        v[:], fro[:], -VAR_MARGIN, 0.0, mybir.AluOpType.add, mybir.AluOpType.max
    )
    nc.vector.tensor_tensor(v[:], v[:], v[:], mybir.AluOpType.mult)
    ones_k = fin_pool.tile([K, 1], FP32)
    nc.vector.memset(ones_k[:], 1.0)
    vsum_ps = psum_pool.tile([1, 1], FP32)
    nc.tensor.matmul(vsum_ps[:], ones_k[:], v[:], start=True, stop=True)
    vsum = fin_pool.tile([1, 1], FP32)
    nc.vector.tensor_copy(vsum[:], vsum_ps[:])

    # centroid^T via DVE 32x32 block transpose: [32,32] -> rows 0:16 hold [16,32]
    cw = fin_pool.tile([K, K], FP32)
    nc.vector.memset(cw[:], 0.0)
    nc.vector.tensor_copy(cw[:, 0:NCH], centroid[:])
    ct = fin_pool.tile([K, K], FP32)
    nc.vector.transpose(ct[:], cw[:])

    # diff = ct - roll(ct, 1, axis=free) over label axis (cols), rows 0:16
    diff = fin_pool.tile([NCH, K], FP32)
    nc.vector.tensor_tensor(
        diff[:, 1:K], ct[0:NCH, 1:K], ct[0:NCH, 0 : K - 1], mybir.AluOpType.subtract
    )
    nc.vector.tensor_tensor(
        diff[:, 0:1], ct[0:NCH, 0:1], ct[0:NCH, K - 1 : K], mybir.AluOpType.subtract
    )

    d2 = fin_pool.tile([NCH, K], FP32)
    nc.vector.tensor_tensor(d2[:], diff[:], diff[:], mybir.AluOpType.mult)
    dsq_ps = psum_pool.tile([1, K], FP32)
    nc.tensor.matmul(dsq_ps[:], ones_k[0:NCH, :], d2[:], start=True, stop=True)
    dsq = fin_pool.tile([1, K], FP32)
    nc.vector.tensor_copy(dsq[:], dsq_ps[:])

    dist = fin_pool.tile([1, K], FP32)
    nc.scalar.activation(
        dist[:], dsq[:], mybir.ActivationFunctionType.Sqrt, bias=eps_k[0:1, :]
    )
    # hinge = max(margin - dist, 0)^2 ; dis_sum = sum_k
    hg = fin_pool.tile([1, K], FP32)
    nc.vector.tensor_scalar(
        hg[:], dist[:], -1.0, DIST_MARGIN, mybir.AluOpType.mult, mybir.AluOpType.add
    )
    nc.vector.tensor_scalar_max(hg[:], hg[:], 0.0)
    nc.vector.tensor_tensor(hg[:], hg[:], hg[:], mybir.AluOpType.mult)
    dsum = fin_pool.tile([1, 1], FP32)
    nc.vector.tensor_reduce(
        dsum[:], hg[:], axis=mybir.AxisListType.X, op=mybir.AluOpType.add
    )

    out_sb = fin_pool.tile([1, 2], FP32)
    nc.vector.tensor_scalar_mul(out_sb[:, 0:1], vsum[:], 1.0 / K)
    nc.vector.tensor_scalar_mul(out_sb[:, 1:2], dsum[:], 1.0 / K)
    nc.sync.dma_start(out_d[:], out_sb[:])
    stack.close()


_NC_CACHE = {}


def _build(repeat: int = 1):
    if repeat not in _NC_CACHE:
        nc = bacc.Bacc(None, target_bir_lowering=False)
        with tile.TileContext(nc) as tc:
            _pidgeon_kernel(tc, repeat=repeat)
        nc.finalize()
        _NC_CACHE[repeat] = nc
    return _NC_CACHE[repeat]


def kernel(y: np.ndarray, c: np.ndarray) -> np.ndarray:
    assert y.shape == (B, NCH, H, W) and c.shape == (B, 1, H, W)
    nc = _build()
    in_maps = [
        {
            "y": np.ascontiguousarray(y[i].reshape(NCH, P), dtype=np.float32),
            "c": np.ascontiguousarray(c[i, 0].reshape(P), dtype=np.int32),
        }
        for i in range(B)
    ]
    last_err = None
    for _attempt in range(3):
        try:
            res = run_bass_kernel_spmd(nc, in_maps, list(range(B)))
            break
        except Exception as e:  # transient device errors: retry
            last_err = e
    else:
        raise last_err
    outs = np.stack([res.results[i]["out"].reshape(2) for i in range(B)])
    loss = outs[:, 0].mean() + outs[:, 1].mean()
    return np.float32(loss)
